# revision 6
# baseline (speedup 1.0000x reference)
"""Causal multi-head self-attention on 8 Trainium2 NeuronCores.

Sharding: data-parallel over batch (B=2) x tensor-parallel over heads
(16 heads -> 4 per core).  Each core computes, for its batch element and
its 4 heads: Q/K/V projections, causal softmax attention, and a partial
output projection (row-parallel Wo).  The host sums the 4 TP partials per
batch and adds bo.

Per-core structure (all matmul inputs bf16):
  QT/KT are stored pair-packed: pair p holds head 2p in partitions 0-63
  and head 2p+1 in partitions 64-127.  The S^T = K^T.T @ Q^T matmuls then
  run K=64 with the two heads row-tiled into the PE array concurrently
  (tile_position (0,0)/(64,0), auto-derived from the base partitions) -
  no zero-padding of the contraction dim.  PV runs col-tiled (M=64 per
  head at out partitions 0/64 of a shared PSUM bank).  Softmax
  denominators come from M=1 ones-matmuls col-tiled 4 heads deep into one
  PSUM bank (partitions 0/32/64/96).  Each shared-bank accumulation is
  preceded by one K=1 zero-matmul with start=True so every element of the
  bank has defined data + has_written state; the real matmuls then all
  accumulate with start=False.

  exp() runs on the scalar engine over [128, 2x512] PSUM spans (two
  j-tiles per instruction) to amortize the ~352-cycle ACT overhead; for
  the mostly-masked second diagonal step it reads only the live halves.
  Causal masking: S/PV/denominator matmuls are N-trimmed to the live
  column range of diagonal tiles; the per-tile triangle is zeroed post-
  exp by gpsimd affine_select on a [128,128] subrange.

  Projections for token-chunk k+1 and the output projection for chunk
  k-1 are emitted interleaved with attention chunk k so the PE fills the
  gaps of the ACT-bound softmax stream.  PSUM (8 banks): 4 = S/proj ring,
  2 = ctx accumulators, 1 = denominators + 1/d broadcast, 1 = out-proj.
"""
import math

import ml_dtypes
import numpy as np

import concourse.bass as bass
import concourse.mybir as mybir
import concourse.tile as tile
from concourse import bacc
from concourse.bass_utils import run_bass_kernel_spmd

F32 = mybir.dt.float32
BF16 = mybir.dt.bfloat16
AF = mybir.ActivationFunctionType
OP = mybir.AluOpType

B, T, D, H, HD = 2, 2048, 1024, 16, 64
NCORES, TP = 8, 4
HPC = H // TP          # heads per core = 4
CS = HPC * HD          # channel shard per core = 256
SCALE = 1.0 / math.sqrt(HD)
KB = D // 128          # 8 k-blocks of the d contraction
TCH = 512              # i-chunk (queries per attention inner pass)
NI = T // TCH          # 4 i-chunks
NTB = T // 128         # 16 token blocks

_CACHE: dict = {}
LAST_EXEC_NS = None
LAST_RESULTS = None


def _build(has_bias: bool):
    nc = bacc.Bacc("TRN2", target_bir_lowering=False, debug=False,
                   num_devices=NCORES)

    xT_d = nc.dram_tensor("xT", [D, T], BF16, kind="ExternalInput").ap()
    wqT_d = nc.dram_tensor("wqT", [D, CS], BF16, kind="ExternalInput").ap()
    wkT_d = nc.dram_tensor("wkT", [D, CS], BF16, kind="ExternalInput").ap()
    wvT_d = nc.dram_tensor("wvT", [D, CS], BF16, kind="ExternalInput").ap()
    woT_d = nc.dram_tensor("woT", [CS, D], BF16, kind="ExternalInput").ap()
    bq_d = nc.dram_tensor("bq", [1, CS], BF16, kind="ExternalInput").ap()
    bk_d = nc.dram_tensor("bk", [1, CS], BF16, kind="ExternalInput").ap()
    bv_d = nc.dram_tensor("bv", [1, CS], BF16, kind="ExternalInput").ap()
    out_d = nc.dram_tensor("out", [T, D], F32, kind="ExternalOutput").ap()

    xt_view = xT_d.rearrange("(a p) t -> a p t", p=128)

    with tile.TileContext(nc) as tc:
        with (
            tc.tile_pool(name="persist", bufs=1) as pp,
            tc.tile_pool(name="ptp", bufs=8) as ptp,      # P^T sbuf tiles
            tc.tile_pool(name="rop", bufs=4) as rop,      # out staging
            tc.tile_pool(name="rdp", bufs=2) as rdp,      # recip staging
            tc.tile_pool(name="pss", bufs=2, space="PSUM") as pss,   # 4 banks
            tc.tile_pool(name="pcx", bufs=2, space="PSUM") as pcx,   # 2 banks
            tc.tile_pool(name="pdn", bufs=1, space="PSUM") as pdn,   # 1 bank
            tc.tile_pool(name="pro", bufs=1, space="PSUM") as pro,   # 1 bank
        ):
            XT = pp.tile([128, KB, T], BF16)
            WQ = pp.tile([128, KB, CS], BF16)
            WK = pp.tile([128, KB, CS], BF16)
            WV = pp.tile([128, KB, CS], BF16)
            WO = pp.tile([128, 2, D], BF16)
            QT = pp.tile([128, 2, T], BF16)     # pair p: heads 2p|2p+1
            KT = pp.tile([128, 2, T], BF16)
            V = pp.tile([128, NTB, CS], BF16)   # [j-in-tile, jt, head*64+c]
            CT = pp.tile([128, 2, T], BF16)     # normalized ctx^T, pair-major
            ONESB = pp.tile([128, HD], BF16)
            ZROW = pp.tile([1, TCH], BF16)
            if has_bias:
                ONESROW = pp.tile([1, T], BF16)
                BQ = pp.tile([1, CS], BF16)
                BK = pp.tile([1, CS], BF16)
                BV = pp.tile([1, CS], BF16)

            nc.gpsimd.memset(ONESB[:], 1.0)
            nc.gpsimd.memset(ZROW[:], 0.0)
            if has_bias:
                nc.gpsimd.memset(ONESROW[:], 1.0)

            # ---------------- DMA queueing ----------------
            nc.sync.dma_start(out=WK[:], in_=wkT_d.rearrange("(a p) c -> p a c", p=128))
            nc.scalar.dma_start(out=WQ[:], in_=wqT_d.rearrange("(a p) c -> p a c", p=128))
            if has_bias:
                nc.scalar.dma_start(out=BQ[:], in_=bq_d[:])
                nc.scalar.dma_start(out=BK[:], in_=bk_d[:])
                nc.scalar.dma_start(out=BV[:], in_=bv_d[:])
            # x arrives i-chunk-major so chunk-0 projections start early
            for tcn in range(NI):
                tsl = slice(tcn * TCH, (tcn + 1) * TCH)
                for kb in range(KB):
                    eng = nc.sync if kb % 2 == 0 else nc.scalar
                    eng.dma_start(out=XT[:, kb, tsl], in_=xt_view[kb][:, tsl])
                if tcn == 0:
                    nc.sync.dma_start(out=WV[:], in_=wvT_d.rearrange("(a p) c -> p a c", p=128))
                if tcn == 1:
                    nc.scalar.dma_start(out=WO[:], in_=woT_d.rearrange("(a p) o -> p a o", p=128))

            # ---------------- emitters ----------------
            def proj_qk_pair(which, tcn):
                """Q^T/K^T projection, both head-pairs, one token chunk."""
                W_sb, dst, scl = ((WQ, QT, SCALE) if which == "q"
                                  else (WK, KT, 1.0))
                tsl = slice(tcn * TCH, (tcn + 1) * TCH)
                p = pss.tile([128, 2, TCH], F32, tag="ss",
                             name=f"pj_{which}_{tcn}")
                for ob in range(2):
                    for kb in range(KB):
                        nc.tensor.matmul(
                            p[:, ob, :], W_sb[:, kb, ob * 128:(ob + 1) * 128],
                            XT[:, kb, tsl],
                            start=(kb == 0),
                            stop=(kb == KB - 1 and not has_bias))
                    if has_bias:
                        bt = BQ if which == "q" else BK
                        nc.tensor.matmul(
                            p[:, ob, :], bt[0:1, ob * 128:(ob + 1) * 128],
                            ONESROW[0:1, tsl], start=False, stop=True)
                for ob in range(2):
                    if scl == 1.0:
                        nc.vector.tensor_copy(out=dst[:, ob, tsl],
                                              in_=p[:, ob, :])
                    else:
                        nc.vector.tensor_scalar_mul(out=dst[:, ob, tsl],
                                                    in0=p[:, ob, :],
                                                    scalar1=scl)

            def proj_v_pair(tb0):
                """V projection for token blocks tb0, tb0+1 (bank each)."""
                p = pss.tile([128, 2, TCH], F32, tag="ss", name=f"pj_v{tb0}")
                for i, tb in enumerate((tb0, tb0 + 1)):
                    for kb in range(KB):
                        nc.tensor.matmul(
                            p[:, i, 0:CS], XT[:, kb, tb * 128:(tb + 1) * 128],
                            WV[:, kb, :],
                            start=(kb == 0),
                            stop=(kb == KB - 1 and not has_bias))
                    if has_bias:
                        nc.tensor.matmul(
                            p[:, i, 0:CS],
                            ONESROW[0:1, tb * 128:(tb + 1) * 128],
                            BV[0:1, :], start=False, stop=True)
                for i, tb in enumerate((tb0, tb0 + 1)):
                    nc.vector.tensor_copy(out=V[:, tb, :], in_=p[:, i, 0:CS])

            def outproj_into(ps, tb, on):
                for cbk in range(2):
                    nc.tensor.matmul(
                        ps, CT[:, cbk, tb * 128:(tb + 1) * 128],
                        WO[:, cbk, on * TCH:(on + 1) * TCH],
                        start=(cbk == 0), stop=(cbk == 1))
                ob_sb = rop.tile([128, TCH], F32, tag="ro")
                nc.vector.tensor_copy(out=ob_sb[:], in_=ps)
                nc.sync.dma_start(
                    out=out_d[tb * 128:(tb + 1) * 128,
                              on * TCH:(on + 1) * TCH],
                    in_=ob_sb[:])

            def outproj_chunk(tb, on):
                ps = pro.tile([128, TCH], F32, tag="pro",
                              name=f"po_{tb}_{on}")
                outproj_into(ps[:], tb, on)

            fillers = []

            def drain(n):
                for _ in range(min(n, len(fillers))):
                    fillers.pop(0)()

            # prologue: chunk-0 projections + V token blocks 0-3
            proj_qk_pair("k", 0)
            proj_qk_pair("q", 0)
            proj_v_pair(0)
            proj_v_pair(2)

            # ---------------- attention + pipelined tails ----------------
            for icn in range(NI):
                isl = slice(icn * TCH, (icn + 1) * TCH)
                jt_max = (icn + 1) * 4
                nsteps = jt_max // 2

                if icn + 1 < NI:
                    fillers.append(lambda t=icn + 1: proj_qk_pair("q", t))
                    fillers.append(lambda t=icn + 1: proj_qk_pair("k", t))
                    fillers.append(lambda b=4 * icn + 4: proj_v_pair(b))
                    fillers.append(lambda b=4 * icn + 6: proj_v_pair(b))
                if icn >= 1:
                    for tb in range(4 * icn - 4, 4 * icn):
                        for on in range(2):
                            fillers.append(
                                lambda tb=tb, on=on: outproj_chunk(tb, on))
                ndrain = {0: 2, 1: 3, 2: 2, 3: 1}[icn]

                dn = pdn.tile([128, TCH], F32, tag="pdn", name=f"dn{icn}")
                pctx = [pcx.tile([128, TCH], F32, tag="pcx",
                                 name=f"pctx{icn}_{p}") for p in range(2)]
                # zero-matmuls: define every element + has_written bit of
                # the shared banks so the col-tiled groups below can all
                # accumulate with start=False under either clear semantics.
                nc.tensor.matmul(dn[:], ZROW[0:1, 0:128], ZROW[0:1, :],
                                 start=True, stop=False, skip_group_check=True)
                for p in range(2):
                    nc.tensor.matmul(pctx[p][:], ZROW[0:1, 0:128],
                                     ZROW[0:1, :], start=True, stop=False,
                                     skip_group_check=True)

                for s in range(nsteps):
                    jts = (2 * s, 2 * s + 1)
                    diag = [max(0, (jt - 4 * icn) * 128) for jt in jts]
                    pts = {}
                    for pr in range(2):
                        SS = [pss.tile([128, 2, TCH], F32, tag="ss",
                                       name=f"ss{icn}_{s}_{pr}_{hh}")
                              for hh in range(2)]
                        # S^T: two heads row-tiled (K=64 each), per j-tile
                        for q, jt in enumerate(jts):
                            e = diag[q]
                            jsl = slice(jt * 128, (jt + 1) * 128)
                            for hh in range(2):
                                rows = slice(64 * hh, 64 * hh + 64)
                                nc.tensor.matmul(
                                    SS[hh][:, q, e:],
                                    KT[rows, pr, jsl],
                                    QT[rows, pr, isl][:, e:],
                                    start=True, stop=True)
                        # exp: one ACT instruction per head over both
                        # j-tiles; the m={2,3} diagonal step reads only
                        # the live column halves.
                        for hh in range(2):
                            pt = ptp.tile([128, 2, TCH], BF16, tag="pt",
                                          name=f"pt{icn}_{s}_{pr}_{hh}")
                            if diag[0] >= 256:
                                nc.scalar.activation(
                                    pt[:, :, 256:], SS[hh][:, :, 256:],
                                    AF.Exp)
                            else:
                                nc.scalar.activation(pt[:], SS[hh][:],
                                                     AF.Exp)
                            pts[(pr, hh)] = pt
                        # causal triangle zeroing on diagonal j-tiles
                        for q, jt in enumerate(jts):
                            e = diag[q]
                            if jt >= 4 * icn:
                                for hh in range(2):
                                    seg = pts[(pr, hh)][:, q, e:e + 128]
                                    nc.gpsimd.affine_select(
                                        out=seg, in_=seg,
                                        compare_op=OP.is_ge, fill=0.0,
                                        base=0, channel_multiplier=-1,
                                        pattern=[[1, 128]])
                        # PV: two heads col-tiled into the pair's ctx bank
                        for q, jt in enumerate(jts):
                            e = diag[q]
                            for hh in range(2):
                                h = 2 * pr + hh
                                nc.tensor.matmul(
                                    pctx[pr][64 * hh:64 * hh + 64, e:],
                                    V[:, jt, 64 * h:64 * h + 64],
                                    pts[(pr, hh)][:, q, e:],
                                    start=False,
                                    stop=(jt == jt_max - 1),
                                    skip_group_check=True)
                    # denominators: 4 heads col-tiled M=1 into one bank
                    for q, jt in enumerate(jts):
                        e = diag[q]
                        for pr in range(2):
                            for hh in range(2):
                                h = 2 * pr + hh
                                nc.tensor.matmul(
                                    dn[32 * h:32 * h + 1, e:],
                                    ONESB[0:128, 0:1],
                                    pts[(pr, hh)][:, q, e:],
                                    start=False,
                                    stop=(jt == jt_max - 1),
                                    skip_group_check=True,
                                    tile_position=(0, 32 * h))
                    drain(ndrain)

                # normalization: 1/denom broadcast over the 64 ctx rows of
                # each head via K=1 ones-matmuls (rhs row = head's denom row)
                rdn = rdp.tile([128, TCH], F32, tag="rdn")
                with nc.allow_low_precision(reason="softmax denom"):
                    nc.vector.reciprocal_approx_fast(out=rdn[:], in_=dn[:])
                rdnb = rdp.tile([128, TCH], BF16, tag="rdnb")
                nc.vector.tensor_copy(out=rdnb[:], in_=rdn[:])
                for pr in range(2):
                    pb = pdn.tile([128, TCH], F32, tag="pdn",
                                  name=f"pb{icn}_{pr}")
                    for hh in range(2):
                        r = 64 * pr + 32 * hh
                        nc.tensor.matmul(
                            pb[64 * hh:64 * hh + 64, :],
                            ONESB[r:r + 1, 0:64],
                            rdnb[r:r + 1, :],
                            start=True, stop=True,
                            tile_position=(r, 64 * hh))
                    pbs = rdp.tile([128, TCH], BF16, tag="pbs",
                                   name=f"pbs{icn}_{pr}")
                    nc.vector.tensor_copy(out=pbs[:], in_=pb[:])
                    nc.vector.tensor_tensor(
                        out=CT[:, pr, isl], in0=pctx[pr][:], in1=pbs[:],
                        op=OP.mult)

            # epilogue: leftover fillers, then the last chunk's output
            # projection batched 2-chunks-per-tile through the freed S banks
            drain(len(fillers))
            chunks = [(tb, on) for tb in range(4 * NI - 4, 4 * NI)
                      for on in range(2)]
            for c0 in range(0, len(chunks), 2):
                ps = pss.tile([128, 2, TCH], F32, tag="ss", name=f"poe{c0}")
                for i, (tb, on) in enumerate(chunks[c0:c0 + 2]):
                    outproj_into(ps[:, i, :], tb, on)

    nc.compile()
    return nc


def _get_nc(has_bias: bool):
    key = ("nc", has_bias)
    if key not in _CACHE:
        _CACHE[key] = _build(has_bias)
    return _CACHE[key]


def _maybe_wire_ntff_hook():
    try:
        import antenv.axon_hooks  # noqa: F401  already present
        return
    except ImportError:
        pass
    try:
        import sys, types
        import trn_agent_boot.trn_boot as boot
        hook = boot._ntff_profile_via_ctypes("/opt/axon/libaxon_pjrt.so")
        mod = types.ModuleType("antenv.axon_hooks")
        mod.get_axon_ntff_profile_hook = lambda: hook
        mod.set_axon_ntff_profile_hook = lambda h: None
        sys.modules["antenv.axon_hooks"] = mod
    except Exception:
        pass


def kernel(x, Wq, bq, Wk, bk, Wv, bv, Wo, bo, _trace=False):
    global LAST_EXEC_NS, LAST_RESULTS
    x = np.asarray(x, np.float32)
    Wq = np.asarray(Wq, np.float32); bq = np.asarray(bq, np.float32)
    Wk = np.asarray(Wk, np.float32); bk = np.asarray(bk, np.float32)
    Wv = np.asarray(Wv, np.float32); bv = np.asarray(bv, np.float32)
    Wo = np.asarray(Wo, np.float32); bo = np.asarray(bo, np.float32)

    has_bias = bool(np.any(bq) or np.any(bk) or np.any(bv))
    nc = _get_nc(has_bias)

    BFNP = ml_dtypes.bfloat16
    xTs = [np.ascontiguousarray(x[b].T).astype(BFNP) for b in range(B)]

    in_maps = []
    for c in range(NCORES):
        b, tpr = divmod(c, TP)
        rows = slice(CS * tpr, CS * (tpr + 1))
        in_maps.append({
            "xT": xTs[b],
            "wqT": np.ascontiguousarray(Wq[rows, :].T).astype(BFNP),
            "wkT": np.ascontiguousarray(Wk[rows, :].T).astype(BFNP),
            "wvT": np.ascontiguousarray(Wv[rows, :].T).astype(BFNP),
            "woT": np.ascontiguousarray(Wo[:, rows].T).astype(BFNP),
            "bq": np.ascontiguousarray(bq[rows]).reshape(1, CS).astype(BFNP),
            "bk": np.ascontiguousarray(bk[rows]).reshape(1, CS).astype(BFNP),
            "bv": np.ascontiguousarray(bv[rows]).reshape(1, CS).astype(BFNP),
        })

    if _trace:
        _maybe_wire_ntff_hook()
    res = run_bass_kernel_spmd(nc, in_maps, core_ids=list(range(NCORES)),
                               trace=bool(_trace))
    LAST_EXEC_NS = res.exec_time_ns
    LAST_RESULTS = res

    out = np.empty((B, T, D), np.float32)
    for b in range(B):
        acc = res.results[TP * b]["out"].astype(np.float32)
        for tpr in range(1, TP):
            acc = acc + res.results[TP * b + tpr]["out"]
        out[b] = acc + bo[None, :]
    return out
